# revision 34
# baseline (speedup 1.0000x reference)
"""AugmentedTripletLoss Trainium2 kernel — 8-core SPMD, row-sharded, v3 (fp8).

Math (matches reference):
  d2[i,j] = sq_i + sq_j - 2*S_ij,  S = X@X.T
  ap_i = sqrt(clip(max_{same}(d2), 1e-12));  an_i from min over diff-class
  plus prototype (normalized-center) augmentation; loss = mean(relu(1+ap-an)).

Device strategy (per core, 512 query rows of the class-SORTED order):
  Host sorts rows by class, rounds X to fp8e4m3 (sq computed from the SAME
  rounded values, so d2 = ||x~i - x~j||^2 exactly), and rolls the key axis
  per core so the core's queries sit at local key columns [128, 640).
  One GEMM with augmented contraction 896 = 768 (X^T, fp8 DoubleRow, 3
  tiles of 256) + 128 (bf16 mask/sq rows) computes
     w = S - sq_j/2 - sq_i/2 - (BIG/2)*[same class]
  directly in PSUM, so  -2w = d2 + BIG*[same]:
    an2 = -2*max_j w                     (same-class pushed away by BIG)
    ap2 = -2*min_{window} w - BIG
  where the per-m-tile window [m*128, m*128+384) is compile-time fixed
  thanks to the roll (covers any class block of size <= 128).
  Centers ride the same stationaries; epilogue sqrt/relu on [128,4] tiles;
  per-core [128,4] partials summed on host. No transposes, casts, or
  collectives on device.
"""
import sys

for _p in ("/opt/trn_rl_repo", "/root/.axon_site"):
    if _p not in sys.path:
        sys.path.insert(0, _p)

import numpy as np
import ml_dtypes

import concourse.bass as bass
import concourse.bacc as bacc
import concourse.mybir as mybir
from concourse.tile import TileContext
from concourse.bass_utils import run_bass_kernel_spmd

F32 = mybir.dt.float32
BF16 = mybir.dt.bfloat16
F8 = mybir.dt.float8e4
ALU = mybir.AluOpType
ACTF = mybir.ActivationFunctionType
AX = mybir.AxisListType
DR = mybir.MatmulPerfMode.DoubleRow

N_CORES = 8
N, D, P = 4096, 768, 100
NQ = N // N_CORES              # 512 query rows per core
MQ = NQ // 128                 # 4 query m-tiles
NB = 3                         # fp8 DoubleRow contraction tiles of 256
NJ = N // 512                  # 8 key column groups of 512
BIG = 16384.0
MARGIN = 1.0
BF = ml_dtypes.bfloat16
F8NP = ml_dtypes.float8_e4m3

_nc_cache = None


def _build():
    nc = bacc.Bacc("TRN2", target_bir_lowering=False, num_devices=N_CORES)

    kt8_h = nc.declare_dram_parameter("kt8", [NB * 128, 2 * N], F8, isOutput=False)
    kt6_h = nc.declare_dram_parameter("kt6", [128, N], BF16, isOutput=False)
    qt6_h = nc.declare_dram_parameter("qt6", [128, NQ], BF16, isOutput=False)
    ct8_h = nc.declare_dram_parameter("ct8", [NB * 128, 256], F8, isOutput=False)
    ct6_h = nc.declare_dram_parameter("ct6", [128, 128], BF16, isOutput=False)
    lvec_h = nc.declare_dram_parameter("lvec", [128, MQ], F32, isOutput=True)

    with TileContext(nc) as tc:
        from contextlib import ExitStack

        with ExitStack() as ctx:
            const = ctx.enter_context(tc.tile_pool(name="const", bufs=1))
            pmain = ctx.enter_context(tc.tile_pool(name="pmain", bufs=6, space="PSUM"))
            pcen = ctx.enter_context(tc.tile_pool(name="pcen", bufs=1, space="PSUM"))

            # ---------- persistent SBUF operands ----------
            kT8 = [const.tile([128, 2, N], F8, tag=f"kT8{b}", name=f"kT8{b}")
                   for b in range(NB)]
            kT6 = const.tile([128, N], BF16, tag="kT6")
            qt6 = const.tile([128, NQ], BF16, tag="qt6")
            cT8 = [const.tile([128, 2, 128], F8, tag=f"cT8{b}", name=f"cT8{b}")
                   for b in range(NB)]
            cT6 = const.tile([128, 128], BF16, tag="cT6")

            # ---------- input DMAs ----------
            # kt8 on sync(b0,b2)/scalar(b1) HW-DGE queues; kT6 on scalar;
            # qt6 leads the scalar queue; center operands ride the otherwise
            # idle gpsimd software-DGE queue.
            nc.scalar.dma_start(out=qt6[:], in_=qt6_h[:, :])

            def kt8_dma(eng, b, c0, c1):
                eng.dma_start(
                    out=kT8[b][:, :, c0:c1],
                    in_=bass.AP(
                        tensor=kt8_h,
                        offset=(b * 128) * (2 * N) + c0,
                        ap=[[2 * N, 128], [N, 2], [1, c1 - c0]],
                    ),
                )

            # chunks 0-2 back-to-back on the HW-DGE queues (they pace the early
            # stream); centers + chunks 3-4 on gpsimd, whose slower software
            # issue rate doesn't matter by the time the PE reaches jj4+.
            for ci, (c0, c1) in enumerate([(0, 512), (512, 1024), (1024, 2048)]):
                for b in range(NB):
                    kt8_dma(nc.sync if (b % 2 == 0) else nc.scalar, b, c0, c1)
                nc.scalar.dma_start(out=kT6[:, c0:c1], in_=kt6_h[:, c0:c1])
            for b in range(NB):
                nc.gpsimd.dma_start(
                    out=cT8[b][:, :, :], in_=ct8_h[b * 128 : (b + 1) * 128, :]
                )
            nc.gpsimd.dma_start(out=cT6[:], in_=ct6_h[:, :])
            for (c0, c1) in [(2048, 3072), (3072, 4096)]:
                for b in range(NB):
                    kt8_dma(nc.gpsimd, b, c0, c1)
                nc.gpsimd.dma_start(out=kT6[:, c0:c1], in_=kt6_h[:, c0:c1])

            # PE pre-warm: dummy matmuls during the input-DMA wait window ramp
            # the tensor engine's p-state (measured: first ~8us of the stream
            # otherwise runs at ~2x slow mid-state) so the real stream starts
            # near full clock. Output goes to a never-read PSUM tile.
            dzero = const.tile([128, 512], BF16, tag="dzero")
            nc.vector.memset(dzero[:], 0.0)
            for _w in range(8):
                pw = pmain.tile([128, 512], F32, tag="mm")
                nc.tensor.matmul(
                    pw[:], dzero[:, 0:128], dzero[:], start=True, stop=True
                )

            def mm_group(pt, m, rhs8, rhs6, n8, n6):
                ms = slice(128 + m * 128, 256 + m * 128)
                for b in range(NB):
                    nc.tensor.matmul(
                        pt[:, 0:n6], kT8[b][:, :, ms], rhs8(b, n8),
                        start=(b == 0), stop=False, perf_mode=DR,
                    )
                nc.tensor.matmul(
                    pt[:, 0:n6], qt6[:, m * 128 : (m + 1) * 128], rhs6(n6),
                    start=False, stop=True,
                )

            # ---------- accumulators ----------
            ancols = [const.tile([128, NJ], F32, name=f"ancols{m}") for m in range(MQ)]
            apw = const.tile([128, 2 * MQ], F32, tag="apw")
            nc.vector.memset(apw[:], 3.0e38)
            cmax = const.tile([128, MQ], F32, tag="cmax")
            anmax = const.tile([128, MQ], F32, tag="anmax")
            apmin = const.tile([128, MQ], F32, tag="apmin")
            epin = const.tile([128, 3 * MQ], F32, tag="epin")
            epd = const.tile([128, 3 * MQ], F32, tag="epd")

            # window partials: m -> [(jj, lo, hi, slot)]
            wparts = {0: [(0, 0, 384, 0)],
                      1: [(0, 128, 512, 0)],
                      2: [(0, 256, 512, 0), (1, 0, 128, 1)],
                      3: [(0, 384, 512, 0), (1, 0, 256, 1)]}

            def main_tile(jj, m):
                js = slice(jj * 512, (jj + 1) * 512)
                pt = pmain.tile([128, 512], F32, tag="mm")
                mm_group(pt, m,
                         rhs8=lambda b, n: kT8[b][:, :, js],
                         rhs6=lambda n: kT6[:, js], n8=512, n6=512)
                nc.vector.tensor_reduce(
                    out=ancols[m][:, jj : jj + 1], in_=pt[:], axis=AX.X, op=ALU.max
                )
                for (wjj, lo, hi, slot) in wparts[m]:
                    if wjj == jj:
                        nc.vector.tensor_reduce(
                            out=apw[:, 2 * m + slot : 2 * m + slot + 1],
                            in_=pt[:, lo:hi], axis=AX.X, op=ALU.min,
                        )

            # ---------- tensor stream ----------
            # m3's stationary lives at cols [512,640) (chunk 1); the tile
            # framework merges semaphore waits across same-jj group runs, so
            # keep the first groups' deps within chunk 0 and push (jj,3)
            # tiles later.
            for (jj, m) in [(0, 0), (0, 1), (0, 2), (1, 0), (1, 1), (1, 2),
                            (0, 3), (1, 3)]:
                main_tile(jj, m)
            # centers (stationaries in cols [128,640) = chunks 0-1)
            for m in range(MQ):
                pc = pcen.tile([128, P], F32, tag="cen")
                mm_group(pc, m,
                         rhs8=lambda b, n: cT8[b][:, :, 0:P],
                         rhs6=lambda n: cT6[:, 0:P], n8=P, n6=P)
                nc.vector.tensor_reduce(
                    out=cmax[:, m : m + 1], in_=pc[:], axis=AX.X, op=ALU.max
                )

            # early epilogue pieces: ap2 and dc2 columns of epin
            for m in range(MQ):
                nc.vector.tensor_reduce(
                    out=apmin[:, m : m + 1], in_=apw[:, 2 * m : 2 * m + 2],
                    axis=AX.X, op=ALU.min,
                )
            nc.vector.tensor_scalar(
                out=epin[:, 0:MQ], in0=apmin[:], scalar1=-2.0, scalar2=-BIG,
                op0=ALU.mult, op1=ALU.add,
            )
            nc.vector.tensor_scalar_max(epin[:, 0:MQ], epin[:, 0:MQ], 1e-12)
            nc.vector.tensor_scalar_mul(epin[:, 2 * MQ : 3 * MQ], cmax[:], -2.0)
            nc.vector.tensor_scalar_max(
                epin[:, 2 * MQ : 3 * MQ], epin[:, 2 * MQ : 3 * MQ], 0.0
            )

            for jj in range(2, NJ):
                for m in range(MQ):
                    main_tile(jj, m)
                    if jj == NJ - 1:
                        nc.vector.tensor_reduce(
                            out=anmax[:, m : m + 1], in_=ancols[m][:],
                            axis=AX.X, op=ALU.max,
                        )

            # ---------- tail epilogue ----------
            # an2 = clip(-2*anmax, 1e-12); fold min(an2, dc2) BEFORE the sqrt
            # (sqrt is monotone; the degenerate sub-1e-12 cases don't occur)
            nc.vector.tensor_scalar(
                out=epin[:, MQ : 2 * MQ], in0=anmax[:], scalar1=-2.0, scalar2=1e-12,
                op0=ALU.mult, op1=ALU.max,
            )
            nc.vector.tensor_tensor(
                out=epin[:, MQ : 2 * MQ], in0=epin[:, MQ : 2 * MQ],
                in1=epin[:, 2 * MQ : 3 * MQ], op=ALU.min,
            )
            nc.scalar.activation(
                out=epd[:, 0 : 2 * MQ], in_=epin[:, 0 : 2 * MQ], func=ACTF.Sqrt
            )
            diff = const.tile([128, MQ], F32)
            nc.vector.tensor_sub(diff[:], epd[:, 0:MQ], epd[:, MQ : 2 * MQ])
            lvec = const.tile([128, MQ], F32)
            # relu(margin + diff) on DVE to avoid a tail engine switch
            nc.vector.tensor_scalar(
                out=lvec[:], in0=diff[:], scalar1=MARGIN, scalar2=0.0,
                op0=ALU.add, op1=ALU.max,
            )

            nc.sync.dma_start(out=lvec_h[:, :], in_=lvec[:])

    nc.finalize()
    return nc


def _get_nc():
    global _nc_cache
    if _nc_cache is None:
        _nc_cache = _build()
    return _nc_cache


def _hilo16(v):
    hi = v.astype(BF)
    lo = (v - hi.astype(np.float32)).astype(BF)
    return hi.astype(np.float32), lo.astype(np.float32)


def _crow(c):
    # partition row for class c; rows 96-99 hold the sq_j / sq_i payloads
    return c if c < 96 else c + 4


def _in_maps(inputs, targets, center):
    x = np.asarray(inputs, dtype=np.float32)
    tgt = np.asarray(targets).astype(np.int64)
    cen = np.asarray(center, dtype=np.float32)
    assert x.shape == (N, D) and tgt.shape == (N,) and cen.shape == (P, D)

    order = np.argsort(tgt, kind="stable")
    xs = x[order]
    ts = tgt[order]
    sizes = np.bincount(tgt, minlength=P)
    assert sizes.max() <= 128, "class block exceeds ap window"

    x8 = xs.astype(F8NP)                         # rounded once, used everywhere
    xf = x8.astype(np.float32)
    sq = (xf * xf).sum(1)                        # consistent with device GEMM

    cn = cen / np.linalg.norm(cen, axis=1, keepdims=True)
    cn8 = cn.astype(F8NP)
    cnf = cn8.astype(np.float32)
    csq = (cnf * cnf).sum(1)

    mhi, mlo = _hilo16(-0.5 * sq)                # key-side -sq_j/2 rows
    chi, clo = _hilo16(-0.5 * csq)

    # center fp8 operand [384, 256]: [(b,p), (i,cc)] = cn8[cc, 256b+128i+p]
    ct8 = np.zeros((NB, 2, 128, 128), dtype=F8NP)
    cnT = np.ascontiguousarray(cn8.T)            # [768, 100]
    ct8[:, :, :, 0:P] = cnT.reshape(NB, 2, 128, P)
    ct8 = np.ascontiguousarray(ct8.transpose(0, 2, 1, 3).reshape(NB * 128, 256))

    ct6 = np.zeros((128, 128), dtype=np.float32)
    ct6[96, 0:P] = chi
    ct6[97, 0:P] = clo
    ct6[98, 0:P] = 1.0
    ct6[99, 0:P] = 1.0
    ct6 = ct6.astype(BF)

    maps = []
    for c in range(N_CORES):
        R0 = c * NQ
        roll = -(R0 - 128)                       # local col j = global R0-128+j
        xk8 = np.roll(x8, roll, axis=0)          # [N, D] rolled fp8 keys
        tk = np.roll(ts, roll, axis=0)
        mh = np.roll(mhi, roll, 0)
        ml = np.roll(mlo, roll, 0)

        xkT = np.ascontiguousarray(xk8.T)        # [768, 4096]
        kt8 = np.ascontiguousarray(
            xkT.reshape(NB, 2, 128, N).transpose(0, 2, 1, 3).reshape(NB * 128, 2 * N)
        )

        kt6 = np.zeros((128, N), dtype=np.float32)
        for cc in range(P):
            kt6[_crow(cc)] = np.where(tk == cc, -0.5 * BIG, 0.0)
        kt6[96] = mh
        kt6[97] = ml
        kt6[98] = 1.0
        kt6[99] = 1.0
        kt6 = kt6.astype(BF)

        tq = ts[R0 : R0 + NQ]
        qhi, qlo = _hilo16(-0.5 * sq[R0 : R0 + NQ])
        q6 = np.zeros((128, NQ), dtype=np.float32)
        for cc in range(P):
            q6[_crow(cc)] = (tq == cc).astype(np.float32)
        q6[96] = 1.0
        q6[97] = 1.0
        q6[98] = qhi
        q6[99] = qlo
        q6 = q6.astype(BF)

        maps.append({
            "kt8": kt8,
            "kt6": np.ascontiguousarray(kt6),
            "qt6": np.ascontiguousarray(q6),
            "ct8": ct8,
            "ct6": np.ascontiguousarray(ct6),
        })
    return maps


def run(inputs, targets, center, trace=False):
    nc = _get_nc()
    res = run_bass_kernel_spmd(
        nc, _in_maps(inputs, targets, center), list(range(N_CORES)), trace=trace
    )
    total = 0.0
    for r in res.results:
        total += float(np.asarray(r["lvec"], dtype=np.float64).sum())
    loss = np.float32(total / N)
    return np.asarray(loss), res


def kernel(inputs, targets, center):
    out, _ = run(inputs, targets, center)
    return out


# revision 36
# speedup vs baseline: 1.1681x; 1.1681x over previous
"""AugmentedTripletLoss Trainium2 kernel — 8-core SPMD, row-sharded, v3 (fp8).

Math (matches reference):
  d2[i,j] = sq_i + sq_j - 2*S_ij,  S = X@X.T
  ap_i = sqrt(clip(max_{same}(d2), 1e-12));  an_i from min over diff-class
  plus prototype (normalized-center) augmentation; loss = mean(relu(1+ap-an)).

Device strategy (per core, 512 query rows of the class-SORTED order):
  Host sorts rows by class, rounds X to fp8e4m3 (sq computed from the SAME
  rounded values, so d2 = ||x~i - x~j||^2 exactly), and rolls the key axis
  per core so the core's queries sit at local key columns [128, 640).
  One GEMM with augmented contraction 896 = 768 (X^T, fp8 DoubleRow, 3
  tiles of 256) + 128 (bf16 mask/sq rows) computes
     w = S - sq_j/2 - sq_i/2 - (BIG/2)*[same class]
  directly in PSUM, so  -2w = d2 + BIG*[same]:
    an2 = -2*max_j w                     (same-class pushed away by BIG)
    ap2 = -2*min_{window} w - BIG
  where the per-m-tile window [m*128, m*128+384) is compile-time fixed
  thanks to the roll (covers any class block of size <= 128).
  Centers ride the same stationaries; epilogue sqrt/relu on [128,4] tiles;
  per-core [128,4] partials summed on host. No transposes, casts, or
  collectives on device.
"""
import sys

for _p in ("/opt/trn_rl_repo", "/root/.axon_site"):
    if _p not in sys.path:
        sys.path.insert(0, _p)

import numpy as np
import ml_dtypes

import concourse.bass as bass
import concourse.bacc as bacc
import concourse.mybir as mybir
from concourse.tile import TileContext
from concourse.bass_utils import run_bass_kernel_spmd

F32 = mybir.dt.float32
BF16 = mybir.dt.bfloat16
F8 = mybir.dt.float8e4
ALU = mybir.AluOpType
ACTF = mybir.ActivationFunctionType
AX = mybir.AxisListType
DR = mybir.MatmulPerfMode.DoubleRow

N_CORES = 8
N, D, P = 4096, 768, 100
NQ = N // N_CORES              # 512 query rows per core
MQ = NQ // 128                 # 4 query m-tiles
NB = 3                         # fp8 DoubleRow contraction tiles of 256
NJ = N // 512                  # 8 key column groups of 512
BIG = 16384.0
MARGIN = 1.0
BF = ml_dtypes.bfloat16
F8NP = ml_dtypes.float8_e4m3

_nc_cache = None


def _build():
    nc = bacc.Bacc("TRN2", target_bir_lowering=False, num_devices=N_CORES)

    kt8_h = nc.declare_dram_parameter("kt8", [NB * 128, 2 * N], F8, isOutput=False)
    kt6_h = nc.declare_dram_parameter("kt6", [128, N], BF16, isOutput=False)
    qt6_h = nc.declare_dram_parameter("qt6", [128, NQ], BF16, isOutput=False)
    ct8_h = nc.declare_dram_parameter("ct8", [NB * 128, 256], F8, isOutput=False)
    ct6_h = nc.declare_dram_parameter("ct6", [128, 128], BF16, isOutput=False)
    lvec_h = nc.declare_dram_parameter("lvec", [128, MQ], F32, isOutput=True)

    with TileContext(nc) as tc:
        from contextlib import ExitStack

        with ExitStack() as ctx:
            const = ctx.enter_context(tc.tile_pool(name="const", bufs=1))
            pmain = ctx.enter_context(tc.tile_pool(name="pmain", bufs=6, space="PSUM"))
            pcen = ctx.enter_context(tc.tile_pool(name="pcen", bufs=1, space="PSUM"))

            # ---------- persistent SBUF operands ----------
            kT8 = [const.tile([128, 2, N], F8, tag=f"kT8{b}", name=f"kT8{b}")
                   for b in range(NB)]
            kT6 = const.tile([128, N], BF16, tag="kT6")
            qt6 = const.tile([128, NQ], BF16, tag="qt6")
            cT8 = [const.tile([128, 2, 128], F8, tag=f"cT8{b}", name=f"cT8{b}")
                   for b in range(NB)]
            cT6 = const.tile([128, 128], BF16, tag="cT6")

            # ---------- input DMAs ----------
            # kt8 on sync(b0,b2)/scalar(b1) HW-DGE queues; kT6 on scalar;
            # qt6 leads the scalar queue; center operands ride the otherwise
            # idle gpsimd software-DGE queue.
            nc.scalar.dma_start(out=qt6[:], in_=qt6_h[:, :])

            def kt8_dma(eng, b, c0, c1):
                eng.dma_start(
                    out=kT8[b][:, :, c0:c1],
                    in_=bass.AP(
                        tensor=kt8_h,
                        offset=(b * 128) * (2 * N) + c0,
                        ap=[[2 * N, 128], [N, 2], [1, c1 - c0]],
                    ),
                )

            chunks = [(0, 512), (512, 1024), (1024, 2048), (2048, 3072), (3072, 4096)]
            for ci, (c0, c1) in enumerate(chunks):
                for b in range(NB):
                    kt8_dma(nc.sync if (b % 2 == 0) else nc.scalar, b, c0, c1)
                nc.scalar.dma_start(out=kT6[:, c0:c1], in_=kt6_h[:, c0:c1])
            for b in range(NB):
                nc.gpsimd.dma_start(
                    out=cT8[b][:, :, :], in_=ct8_h[b * 128 : (b + 1) * 128, :]
                )
            nc.gpsimd.dma_start(out=cT6[:], in_=ct6_h[:, :])

            def mm_group(pt, m, rhs8, rhs6, n8, n6):
                ms = slice(128 + m * 128, 256 + m * 128)
                for b in range(NB):
                    nc.tensor.matmul(
                        pt[:, 0:n6], kT8[b][:, :, ms], rhs8(b, n8),
                        start=(b == 0), stop=False, perf_mode=DR,
                    )
                nc.tensor.matmul(
                    pt[:, 0:n6], qt6[:, m * 128 : (m + 1) * 128], rhs6(n6),
                    start=False, stop=True,
                )

            # ---------- accumulators ----------
            ancols = [const.tile([128, NJ], F32, name=f"ancols{m}") for m in range(MQ)]
            apw = const.tile([128, 2 * MQ], F32, tag="apw")
            nc.vector.memset(apw[:], 3.0e38)
            cmax = const.tile([128, MQ], F32, tag="cmax")
            anmax = const.tile([128, MQ], F32, tag="anmax")
            apmin = const.tile([128, MQ], F32, tag="apmin")
            epin = const.tile([128, 3 * MQ], F32, tag="epin")
            epd = const.tile([128, 3 * MQ], F32, tag="epd")

            # window partials: m -> [(jj, lo, hi, slot)]
            wparts = {0: [(0, 0, 384, 0)],
                      1: [(0, 128, 512, 0)],
                      2: [(0, 256, 512, 0), (1, 0, 128, 1)],
                      3: [(0, 384, 512, 0), (1, 0, 256, 1)]}

            def main_tile(jj, m):
                js = slice(jj * 512, (jj + 1) * 512)
                pt = pmain.tile([128, 512], F32, tag="mm")
                mm_group(pt, m,
                         rhs8=lambda b, n: kT8[b][:, :, js],
                         rhs6=lambda n: kT6[:, js], n8=512, n6=512)
                nc.vector.tensor_reduce(
                    out=ancols[m][:, jj : jj + 1], in_=pt[:], axis=AX.X, op=ALU.max
                )
                for (wjj, lo, hi, slot) in wparts[m]:
                    if wjj == jj:
                        nc.vector.tensor_reduce(
                            out=apw[:, 2 * m + slot : 2 * m + slot + 1],
                            in_=pt[:, lo:hi], axis=AX.X, op=ALU.min,
                        )

            # ---------- tensor stream ----------
            # m3's stationary lives at cols [512,640) (chunk 1); the tile
            # framework merges semaphore waits across same-jj group runs, so
            # keep the first groups' deps within chunk 0 and push (jj,3)
            # tiles later.
            for (jj, m) in [(0, 0), (0, 1), (0, 2), (1, 0), (1, 1), (1, 2),
                            (0, 3), (1, 3)]:
                main_tile(jj, m)
            # centers (stationaries in cols [128,640) = chunks 0-1)
            for m in range(MQ):
                pc = pcen.tile([128, P], F32, tag="cen")
                mm_group(pc, m,
                         rhs8=lambda b, n: cT8[b][:, :, 0:P],
                         rhs6=lambda n: cT6[:, 0:P], n8=P, n6=P)
                nc.vector.tensor_reduce(
                    out=cmax[:, m : m + 1], in_=pc[:], axis=AX.X, op=ALU.max
                )

            # early epilogue pieces: ap2 and dc2 columns of epin
            for m in range(MQ):
                nc.vector.tensor_reduce(
                    out=apmin[:, m : m + 1], in_=apw[:, 2 * m : 2 * m + 2],
                    axis=AX.X, op=ALU.min,
                )
            nc.vector.tensor_scalar(
                out=epin[:, 0:MQ], in0=apmin[:], scalar1=-2.0, scalar2=-BIG,
                op0=ALU.mult, op1=ALU.add,
            )
            nc.vector.tensor_scalar_max(epin[:, 0:MQ], epin[:, 0:MQ], 1e-12)
            nc.vector.tensor_scalar_mul(epin[:, 2 * MQ : 3 * MQ], cmax[:], -2.0)
            nc.vector.tensor_scalar_max(
                epin[:, 2 * MQ : 3 * MQ], epin[:, 2 * MQ : 3 * MQ], 0.0
            )

            for jj in range(2, NJ):
                for m in range(MQ):
                    main_tile(jj, m)
                    if jj == NJ - 1:
                        nc.vector.tensor_reduce(
                            out=anmax[:, m : m + 1], in_=ancols[m][:],
                            axis=AX.X, op=ALU.max,
                        )

            # ---------- tail epilogue ----------
            # an2 = clip(-2*anmax, 1e-12); fold min(an2, dc2) BEFORE the sqrt
            # (sqrt is monotone; the degenerate sub-1e-12 cases don't occur)
            nc.vector.tensor_scalar(
                out=epin[:, MQ : 2 * MQ], in0=anmax[:], scalar1=-2.0, scalar2=1e-12,
                op0=ALU.mult, op1=ALU.max,
            )
            nc.vector.tensor_tensor(
                out=epin[:, MQ : 2 * MQ], in0=epin[:, MQ : 2 * MQ],
                in1=epin[:, 2 * MQ : 3 * MQ], op=ALU.min,
            )
            nc.scalar.activation(
                out=epd[:, 0 : 2 * MQ], in_=epin[:, 0 : 2 * MQ], func=ACTF.Sqrt
            )
            diff = const.tile([128, MQ], F32)
            nc.vector.tensor_sub(diff[:], epd[:, 0:MQ], epd[:, MQ : 2 * MQ])
            lvec = const.tile([128, MQ], F32)
            # relu(margin + diff) on DVE to avoid a tail engine switch
            nc.vector.tensor_scalar(
                out=lvec[:], in0=diff[:], scalar1=MARGIN, scalar2=0.0,
                op0=ALU.add, op1=ALU.max,
            )

            nc.sync.dma_start(out=lvec_h[:, :], in_=lvec[:])

    nc.finalize()
    return nc


def _get_nc():
    global _nc_cache
    if _nc_cache is None:
        _nc_cache = _build()
    return _nc_cache


def _hilo16(v):
    hi = v.astype(BF)
    lo = (v - hi.astype(np.float32)).astype(BF)
    return hi.astype(np.float32), lo.astype(np.float32)


def _crow(c):
    # partition row for class c; rows 96-99 hold the sq_j / sq_i payloads
    return c if c < 96 else c + 4


def _in_maps(inputs, targets, center):
    x = np.asarray(inputs, dtype=np.float32)
    tgt = np.asarray(targets).astype(np.int64)
    cen = np.asarray(center, dtype=np.float32)
    assert x.shape == (N, D) and tgt.shape == (N,) and cen.shape == (P, D)

    order = np.argsort(tgt, kind="stable")
    xs = x[order]
    ts = tgt[order]
    sizes = np.bincount(tgt, minlength=P)
    assert sizes.max() <= 128, "class block exceeds ap window"

    x8 = xs.astype(F8NP)                         # rounded once, used everywhere
    xf = x8.astype(np.float32)
    sq = (xf * xf).sum(1)                        # consistent with device GEMM

    cn = cen / np.linalg.norm(cen, axis=1, keepdims=True)
    cn8 = cn.astype(F8NP)
    cnf = cn8.astype(np.float32)
    csq = (cnf * cnf).sum(1)

    mhi, mlo = _hilo16(-0.5 * sq)                # key-side -sq_j/2 rows
    chi, clo = _hilo16(-0.5 * csq)

    # center fp8 operand [384, 256]: [(b,p), (i,cc)] = cn8[cc, 256b+128i+p]
    ct8 = np.zeros((NB, 2, 128, 128), dtype=F8NP)
    cnT = np.ascontiguousarray(cn8.T)            # [768, 100]
    ct8[:, :, :, 0:P] = cnT.reshape(NB, 2, 128, P)
    ct8 = np.ascontiguousarray(ct8.transpose(0, 2, 1, 3).reshape(NB * 128, 256))

    ct6 = np.zeros((128, 128), dtype=np.float32)
    ct6[96, 0:P] = chi
    ct6[97, 0:P] = clo
    ct6[98, 0:P] = 1.0
    ct6[99, 0:P] = 1.0
    ct6 = ct6.astype(BF)

    maps = []
    for c in range(N_CORES):
        R0 = c * NQ
        roll = -(R0 - 128)                       # local col j = global R0-128+j
        xk8 = np.roll(x8, roll, axis=0)          # [N, D] rolled fp8 keys
        tk = np.roll(ts, roll, axis=0)
        mh = np.roll(mhi, roll, 0)
        ml = np.roll(mlo, roll, 0)

        xkT = np.ascontiguousarray(xk8.T)        # [768, 4096]
        kt8 = np.ascontiguousarray(
            xkT.reshape(NB, 2, 128, N).transpose(0, 2, 1, 3).reshape(NB * 128, 2 * N)
        )

        kt6 = np.zeros((128, N), dtype=np.float32)
        for cc in range(P):
            kt6[_crow(cc)] = np.where(tk == cc, -0.5 * BIG, 0.0)
        kt6[96] = mh
        kt6[97] = ml
        kt6[98] = 1.0
        kt6[99] = 1.0
        kt6 = kt6.astype(BF)

        tq = ts[R0 : R0 + NQ]
        qhi, qlo = _hilo16(-0.5 * sq[R0 : R0 + NQ])
        q6 = np.zeros((128, NQ), dtype=np.float32)
        for cc in range(P):
            q6[_crow(cc)] = (tq == cc).astype(np.float32)
        q6[96] = 1.0
        q6[97] = 1.0
        q6[98] = qhi
        q6[99] = qlo
        q6 = q6.astype(BF)

        maps.append({
            "kt8": kt8,
            "kt6": np.ascontiguousarray(kt6),
            "qt6": np.ascontiguousarray(q6),
            "ct8": ct8,
            "ct6": np.ascontiguousarray(ct6),
        })
    return maps


def run(inputs, targets, center, trace=False):
    nc = _get_nc()
    res = run_bass_kernel_spmd(
        nc, _in_maps(inputs, targets, center), list(range(N_CORES)), trace=trace
    )
    total = 0.0
    for r in res.results:
        total += float(np.asarray(r["lvec"], dtype=np.float64).sum())
    loss = np.float32(total / N)
    return np.asarray(loss), res


def kernel(inputs, targets, center):
    out, _ = run(inputs, targets, center)
    return out
